# revision 1
# baseline (speedup 1.0000x reference)
"""Inverse dynamics (RNEA, serial chain of N revolute-z joints) on 8 TRN2 cores.

World-frame reformulation: since every joint rotates about z, rotation
composition reduces to cumulative angles, and the entire forward/backward
recursion becomes segmented cumsums (tensor_tensor_scan with a reset mask)
plus elementwise ops. Pure data-parallel over the batch: B=65536 -> 8192
samples per core, laid out [128 partitions, 64 samples, 32 joints].

    Wc = cumsum(qd), Ac = cumsum(qdd)  (per sample, along joints)
    uP = (Qc_{j-1} + alpha_j)/2pi, uM = (Qc_j + beta_j)/2pi  (host folds
        alpha/beta and the 1/2pi prescale into the scanned inputs)
    P = rp*(cos,sin)(2pi uP), MC = rm*(cos,sin)(2pi uM)
    Ux = cumsum(-Py*WcS), Uy = cumsum(Px*WcS)
    Lx = cumsum(qd*Uy - Py*AcS), Ly = cumsum(Px*AcS - qd*Ux)
    phix = m*Lx + MCy*Ac - m*Wc*Uy + MCx*Wc^2
    phiy = m*Ly - MCx*Ac + m*Wc*Ux + MCy*Wc^2
    gz   = Ibzz*Ac + MCx*Ly - MCy*Lx - Wc*(Ux*MCx + Uy*MCy)
    F    = revcumsum(phi);  e = Px*Fy - Py*Fx
    tau  = revcumsum(gz + e) - e + damping*qd
"""
import numpy as np

B, N = 65536, 32
NCORES = 8
BC = B // NCORES            # 8192 samples per core
P = 128                     # SBUF partitions
S = BC // P                 # 64 samples per partition
CHUNKS = 2
SC = S // CHUNKS            # samples per partition per chunk
CH = SC * N                 # free elems per chunk (1024)
TWO_PI = float(2 * np.pi)
MAGIC = float(1.5 * 2**23)  # fp32 round-to-nearest-even magic constant

_CACHE = {}


def _build_nc(reps=1):
    import concourse.bacc as bacc
    import concourse.bass as bass
    import concourse.mybir as mybir
    from concourse.tile import TileContext

    FP = mybir.dt.float32
    Op = mybir.AluOpType
    Act = mybir.ActivationFunctionType

    nc = bacc.Bacc()

    d_qA = nc.dram_tensor("qA", [BC, N], FP, kind="ExternalInput")
    d_qB = nc.dram_tensor("qB", [BC, N], FP, kind="ExternalInput")
    d_qd = nc.dram_tensor("qd", [BC, N], FP, kind="ExternalInput")
    d_qdd = nc.dram_tensor("qdd", [BC, N], FP, kind="ExternalInput")
    d_msk = nc.dram_tensor("msk", [CH + N], FP, kind="ExternalInput")
    d_cst = nc.dram_tensor("cst", [5, CH], FP, kind="ExternalInput")
    d_tau = nc.dram_tensor("tau", [BC, N], FP, kind="ExternalOutput")

    def dview(d, c):
        # chunk c of a [BC, N] dram tensor as a [P, CH] AP
        return d.ap().rearrange("(p s) j -> p (s j)", p=P)[:, c * CH:(c + 1) * CH]

    with TileContext(nc) as tc:
        with tc.tile_pool(name="cst", bufs=1) as cpool, \
             tc.tile_pool(name="work", bufs=40) as wpool:

            # ---- constants (live whole kernel) ----
            ME = cpool.tile([P, CH + N], FP, tag="ME")
            nc.gpsimd.dma_start(
                out=ME[:, :],
                in_=bass.AP(d_msk, 0, [[0, P], [1, CH + N]]))
            Mf = ME[:, N:]                      # fwd mask: 0 at j=0 of each segment
            Mrev = bass.AP(ME.tensor, CH, [[CH + N, P], [-1, CH]])

            csts = []
            for i, nm in enumerate(["rp", "rm", "mt", "ibz", "dmp"]):
                t = cpool.tile([P, CH], FP, tag=nm, name=nm)
                nc.gpsimd.dma_start(
                    out=t[:, :],
                    in_=bass.AP(d_cst, i * CH, [[0, P], [1, CH]]))
                csts.append(t)
            rp, rm, mt, ibz, dmp = csts

            def rev(t):
                return bass.AP(t.tensor, CH - 1, [[CH, P], [-1, CH]])

            for rep, c in [(r, c) for r in range(reps) for c in range(CHUNKS)]:
                w = {}

                def tile(nm):
                    t = wpool.tile([P, CH], FP, tag="w", name=f"{nm}_{c}_{rep}")
                    w[nm] = t
                    return t

                def load(nm, d):
                    t = tile(nm)
                    nc.gpsimd.dma_start(out=t[:, :], in_=dview(d, c))
                    return t

                qA = load("qA", d_qA)
                qB = load("qB", d_qB)
                qd = load("qd", d_qd)
                qdd = load("qdd", d_qdd)

                def scan(nm, data, op1=Op.add, reverse=False):
                    t = tile(nm)
                    if reverse:
                        nc.vector.tensor_tensor_scan(
                            rev(t), Mrev, rev(data), 0.0, Op.mult, op1)
                    else:
                        nc.vector.tensor_tensor_scan(
                            t[:, :], Mf, data[:, :], 0.0, Op.mult, op1)
                    return t

                def tt(nm, a, b, op):
                    t = tile(nm)
                    nc.vector.tensor_tensor(out=t[:, :], in0=a[:, :], in1=b[:, :], op=op)
                    return t

                def mul(nm, a, b):
                    return tt(nm, a, b, Op.mult)

                def add(nm, a, b):
                    return tt(nm, a, b, Op.add)

                def sub(nm, a, b):
                    return tt(nm, a, b, Op.subtract)

                def trig(pref, u):
                    # u in turns -> sin(2pi u), cos(2pi u) via magic-number
                    # range reduction to [-0.5, 0.5]
                    k = tile(pref + "k")
                    nc.vector.tensor_scalar(k[:, :], u[:, :], MAGIC, MAGIC,
                                            Op.add, Op.subtract)
                    d = tile(pref + "d")
                    nc.vector.scalar_tensor_tensor(d[:, :], k[:, :], -1.0,
                                                   u[:, :], Op.mult, Op.add)
                    s = tile(pref + "s")
                    nc.scalar.activation(s[:, :], d[:, :], Act.Sin, scale=TWO_PI)
                    t2 = tile(pref + "t2")
                    nc.vector.tensor_scalar_add(t2[:, :], u[:, :], 0.25)
                    k2 = tile(pref + "k2")
                    nc.vector.tensor_scalar(k2[:, :], t2[:, :], MAGIC, MAGIC,
                                            Op.add, Op.subtract)
                    d2 = tile(pref + "d2")
                    nc.vector.scalar_tensor_tensor(d2[:, :], k2[:, :], -1.0,
                                                   t2[:, :], Op.mult, Op.add)
                    cs = tile(pref + "c")
                    nc.scalar.activation(cs[:, :], d2[:, :], Act.Sin, scale=TWO_PI)
                    return s, cs

                uP = scan("uP", qA)
                uM = scan("uM", qB)
                Wc = scan("Wc", qd)
                Ac = scan("Ac", qdd)
                WcS = sub("WcS", Wc, qd)
                AcS = sub("AcS", Ac, qdd)

                sP, cP = trig("P", uP)
                sM, cM = trig("M", uM)

                Px = mul("Px", rp, cP)
                Py = mul("Py", rp, sP)
                MCx = mul("MCx", rm, cM)
                MCy = mul("MCy", rm, sM)

                t1 = mul("t1", Py, WcS)
                Ux = scan("Ux", t1, op1=Op.subtract)     # cumsum(-Py*WcS)
                t2_ = mul("t2_", Px, WcS)
                Uy = scan("Uy", t2_)

                g1 = mul("g1", Py, AcS)
                g2 = mul("g2", qd, Uy)
                tLx = sub("tLx", g2, g1)
                Lx = scan("Lx", tLx)
                g3 = mul("g3", Px, AcS)
                g4 = mul("g4", qd, Ux)
                tLy = sub("tLy", g3, g4)
                Ly = scan("Ly", tLy)

                W2 = tile("W2")
                nc.scalar.activation(W2[:, :], Wc[:, :], Act.Square)
                mW = mul("mW", mt, Wc)

                p1 = mul("p1", mt, Lx)
                p2 = mul("p2", MCy, Ac)
                p3 = mul("p3", MCx, W2)
                p4 = mul("p4", mW, Uy)
                s1 = add("s1", p1, p2)
                s2 = sub("s2", p3, p4)
                phix = add("phix", s1, s2)

                q1 = mul("q1", mt, Ly)
                q2 = mul("q2", MCx, Ac)
                q3 = mul("q3", MCy, W2)
                q4 = mul("q4", mW, Ux)
                r1 = sub("r1", q1, q2)
                r2 = add("r2", q3, q4)
                phiy = add("phiy", r1, r2)

                c1 = mul("c1", Ux, MCx)
                c2p = mul("c2p", Uy, MCy)
                c2 = add("c2", c1, c2p)
                c3 = mul("c3", Wc, c2)
                c4 = mul("c4", MCx, Ly)
                c5p = mul("c5p", MCy, Lx)
                c5 = sub("c5", c4, c5p)
                c6 = sub("c6", c5, c3)
                c7 = mul("c7", ibz, Ac)
                gz = add("gz", c6, c7)

                Fx = scan("Fx", phix, reverse=True)
                Fy = scan("Fy", phiy, reverse=True)
                e1 = mul("e1", Px, Fy)
                e1b = mul("e1b", Py, Fx)
                e2 = sub("e2", e1, e1b)
                h = add("h", gz, e2)
                R12 = scan("R12", h, reverse=True)

                dq = mul("dq", dmp, qd)
                tau0 = sub("tau0", R12, e2)
                out = add("out", tau0, dq)
                nc.gpsimd.dma_start(out=dview(d_tau, c), in_=out[:, :])

    nc.finalize()
    return nc


def _host_prep(q, qd, qdd_des, trans, mass, com, inertia, damping):
    px, py = trans[:, 0].astype(np.float64), trans[:, 1].astype(np.float64)
    mc = (mass[:, None] * com).astype(np.float64)
    mcx, mcy = mc[:, 0], mc[:, 1]

    def skew(v):
        x, y, z = v[..., 0], v[..., 1], v[..., 2]
        o = np.zeros_like(x)
        return np.stack([np.stack([o, -z, y], -1),
                         np.stack([z, o, -x], -1),
                         np.stack([-y, x, o], -1)], -2)
    Sk = skew(com.astype(np.float64))
    Ibar = inertia + (mass[:, None, None] * (Sk @ np.swapaxes(Sk, -1, -2))
                      ).astype(np.float32)
    ibzz = Ibar[:, 2, 2].astype(np.float64)

    rp = np.hypot(px, py)
    alpha = np.arctan2(py, px)
    rm = np.hypot(mcx, mcy)
    beta = np.arctan2(mcy, mcx)

    inv2pi = 1.0 / (2 * np.pi)
    # qA: scan -> (Qc_{j-1} + alpha_j)/2pi ; qB: scan -> (Qc_j + beta_j)/2pi
    qA = np.empty((B, N), np.float32)
    qA[:, 0] = np.float32(alpha[0] * inv2pi)
    dal = np.diff(alpha) * inv2pi
    qA[:, 1:] = (q[:, :-1] * np.float32(inv2pi)
                 + dal.astype(np.float32)[None, :])
    qB = np.empty((B, N), np.float32)
    dbe = np.empty(N, np.float64)
    dbe[0] = beta[0]
    dbe[1:] = np.diff(beta)
    qB[:] = q * np.float32(inv2pi) + (dbe * inv2pi).astype(np.float32)[None, :]

    msk = np.ones(CH + N, np.float32)
    msk[N::N] = 0.0

    cst = np.stack([
        np.tile(rp.astype(np.float32), SC),
        np.tile(rm.astype(np.float32), SC),
        np.tile(mass.astype(np.float32), SC),
        np.tile(ibzz.astype(np.float32), SC),
        np.tile(damping.astype(np.float32), SC),
    ]).astype(np.float32)
    return qA, qB, msk, cst


def kernel(q, qd, qdd_des, trans, mass, com, inertia, damping):
    from concourse.bass_utils import run_bass_kernel_spmd

    q = np.asarray(q, np.float32)
    qd = np.asarray(qd, np.float32)
    qdd = np.asarray(qdd_des, np.float32)
    qA, qB, msk, cst = _host_prep(q, qd, qdd, np.asarray(trans), np.asarray(mass),
                                  np.asarray(com), np.asarray(inertia),
                                  np.asarray(damping))
    if "nc" not in _CACHE:
        _CACHE["nc"] = _build_nc()
    nc = _CACHE["nc"]

    in_maps = []
    for cix in range(NCORES):
        sl = slice(cix * BC, (cix + 1) * BC)
        in_maps.append({
            "qA": np.ascontiguousarray(qA[sl]),
            "qB": np.ascontiguousarray(qB[sl]),
            "qd": np.ascontiguousarray(qd[sl]),
            "qdd": np.ascontiguousarray(qdd[sl]),
            "msk": msk,
            "cst": cst,
        })
    res = run_bass_kernel_spmd(nc, in_maps, list(range(NCORES)))
    return np.concatenate([r["tau"] for r in res.results], 0)

